# revision 33
# baseline (speedup 1.0000x reference)
"""AutoCorrelation (Autoformer-style) Trainium2 Bass kernel, v3.

Sharding: data-parallel over batch — 8 batch elements -> 8 NeuronCores, no
collectives. Each core computes its [2048, 128] output slice independently.

Same math as v2; restructured schedule:
  * consumption-ordered per-chunk DMA of the cos/sin DFT tables so the
    forward DFT streams behind the DMA instead of waiting for the full 4MB;
  * QK projections per (freq-half, comp, head); all PSUM evacuations on the
    scalar engine; complex products split per freq-half so the vector engine
    starts ~6us earlier; per-head dedicated SBUF tiles (no pool rotation);
  * single combined acm inverse (all 4 heads accumulated into one pre/pim);
  * V spectra issued after the inverse so they fill the PE/scalar idle
    window under the top-k scans;
  * top-k tail: delay index via fused (eo == v21) * remap_table accumulate
    (one DVE op instead of FIND_INDEX8 + int chain); phase args via two
    dtype-converting tensor_scalar ops (f32 mult -> i32, and -> f32);
    Sin activation table preloaded behind a dummy op; softmax weight and
    -2/T folded into the output projection weights (wo_a / won_a);
  * final combine: op+bo / -op+bo on scalar directly from PSUM, vector adds
    ep; og->f32r copies on the (then idle) vector engine.
"""
import os
import sys
import types
from contextlib import ExitStack

sys.path.insert(0, "/opt/trn_rl_repo")

import numpy as np

import concourse.bass as bass
import concourse.mybir as mybir
from concourse import bacc
from concourse.tile import TileContext
from concourse.bass_utils import run_bass_kernel_spmd

B, T, E, H = 8, 2048, 128, 4
NF = 1024
NCH = 8
AL = mybir.AluOpType
DT = mybir.dt
AF = mybir.ActivationFunctionType
AX = mybir.AxisListType

_CACHE = {}
LAST_EXEC_NS = None


def _wire_ntff_hook():
    if "antenv.axon_hooks" in sys.modules:
        return
    try:
        mod = types.ModuleType("antenv.axon_hooks")
        _h = [None]
        mod.set_axon_ntff_profile_hook = lambda h: _h.__setitem__(0, h)
        mod.get_axon_ntff_profile_hook = lambda: _h[0]
        sys.modules["antenv.axon_hooks"] = mod
        import antenv
        antenv.axon_hooks = mod
        from trn_agent_boot.trn_boot import _ntff_profile_via_ctypes
        mod.set_axon_ntff_profile_hook(_ntff_profile_via_ctypes("/opt/axon/libaxon_pjrt.so"))
    except Exception:
        pass


def _host_consts():
    i = np.arange(NF, dtype=np.float64)
    ang = np.outer(i, i) * (2.0 * np.pi / T)

    def chunk(a):  # [1024,1024] -> [128, 8*1024] chunk-major
        return np.ascontiguousarray(
            a.reshape(NCH, 128, NF).transpose(1, 0, 2).reshape(128, NCH * NF))

    return {
        "cs": chunk(np.cos(ang).astype(np.float16)),
        "sn": chunk(np.sin(ang).astype(np.float16)),
        "altf": ((-1.0) ** np.arange(NF)).astype(np.float16)[None, :],
        "altp": ((-1.0) ** np.arange(128)).astype(np.float16)[:, None],
        "one": np.ones((1, 1), np.float16),
        "mhalf": np.full((1, 1), -0.5, np.float16),
        "mhrow": np.full((1, NF), -0.5, np.float16),
        "ident": np.eye(128, dtype=np.float32),
    }


def _build():
    nc = bacc.Bacc("TRN2", target_bir_lowering=False, debug=False, num_devices=1)
    f32, f32r, f16, i32, u32 = DT.float32, DT.float32r, DT.float16, DT.int32, DT.uint32

    # all 2D tensors ship host-pre-chunked: [partition, chunk-major free]
    xc_d = nc.dram_tensor("xc", [128, NCH * E], f16, kind="ExternalInput")
    xs_d = nc.dram_tensor("xs", [128, NCH * E], f16, kind="ExternalInput")
    xnyq_d = nc.dram_tensor("xnyq", [1, E], f16, kind="ExternalInput")
    cs_d = nc.dram_tensor("cs", [128, NCH * NF], f16, kind="ExternalInput")
    sn_d = nc.dram_tensor("sn", [128, NCH * NF], f16, kind="ExternalInput")
    altf_d = nc.dram_tensor("altf", [1, NF], f16, kind="ExternalInput")
    altp_d = nc.dram_tensor("altp", [128, 1], f16, kind="ExternalInput")
    one_d = nc.dram_tensor("one", [1, 1], f16, kind="ExternalInput")
    mhalf_d = nc.dram_tensor("mhalf", [1, 1], f16, kind="ExternalInput")
    mhrow_d = nc.dram_tensor("mhrow", [1, NF], f16, kind="ExternalInput")
    id_d = nc.dram_tensor("ident", [128, 128], f32r, kind="ExternalInput")
    wqk_d = nc.dram_tensor("wqk", [128, H * 256], f16, kind="ExternalInput")  # pre-scaled 1/32
    wv_d = nc.dram_tensor("wv", [128, H * E], f16, kind="ExternalInput")
    wo_d = nc.dram_tensor("wo", [128, H * E], f16, kind="ExternalInput")
    bqk_d = nc.dram_tensor("bqk", [1, 2 * H * E], f32, kind="ExternalInput")  # interleaved (q_h|k_h)*4
    bv_d = nc.dram_tensor("bv", [E, H], f32, kind="ExternalInput")        # T*bv
    bo_d = nc.dram_tensor("bo", [E, 1], f32, kind="ExternalInput")
    lo_d = nc.dram_tensor("out_lo", [E, NF], f32, kind="ExternalOutput")
    hi_d = nc.dram_tensor("out_hi", [E, NF], f32, kind="ExternalOutput")
    o1024_d = nc.dram_tensor("out_1024", [E, 1], f32, kind="ExternalOutput")

    with TileContext(nc) as tc, ExitStack() as ctx:
        pool = ctx.enter_context(tc.tile_pool(name="main", bufs=1))
        pool2 = ctx.enter_context(tc.tile_pool(name="rot", bufs=2))
        pqk = ctx.enter_context(tc.tile_pool(name="pqk", bufs=2, space="PSUM"))    # [128,1024] tiles
        pb = ctx.enter_context(tc.tile_pool(name="pb", bufs=3, space="PSUM"))      # [128,512] tiles
        prow = ctx.enter_context(tc.tile_pool(name="psrow", bufs=1, space="PSUM"))

        # ---------------- loads, consumption-ordered, split for queue parallelism ----------------
        xnyq = pool.tile([1, E], f16, tag="xnyq")
        nc.sync.dma_start(xnyq[:], xnyq_d[:])
        altf_sb = pool.tile([1, NF], f16, tag="altf")
        nc.sync.dma_start(altf_sb[:], altf_d[:])
        altp_sb = pool.tile([128, 1], f16, tag="altp")
        nc.sync.dma_start(altp_sb[:], altp_d[:])
        one_sb = pool.tile([1, 1], f16, tag="one")
        nc.sync.dma_start(one_sb[:], one_d[:])
        bqk_sb = pool.tile([1, 2 * H * E], f32, tag="bqk")
        nc.sync.dma_start(bqk_sb[:], bqk_d[:])
        xc = pool.tile([128, NCH * E], f16, tag="xc")
        xs = pool.tile([128, NCH * E], f16, tag="xs")
        cs_sb = pool.tile([128, NCH * NF], f16, tag="cs")
        sn_sb = pool.tile([128, NCH * NF], f16, tag="sn")
        for half in range(2):
            nc.sync.dma_start(xc[:, half * 512:(half + 1) * 512],
                              xc_d[:, half * 512:(half + 1) * 512])
        for half in range(2):
            nc.sync.dma_start(xs[:, half * 512:(half + 1) * 512],
                              xs_d[:, half * 512:(half + 1) * 512])
        # s0 table pieces interleaved cs/sn so hre/him pace together
        for a in range(NCH):
            sl = slice(a * NF, a * NF + 512)
            nc.sync.dma_start(cs_sb[:, sl], cs_d[:, sl])
            nc.sync.dma_start(sn_sb[:, sl], sn_d[:, sl])
        wqk_sb = pool.tile([128, H * 256], f16, tag="wqk")
        for half in range(2):
            nc.sync.dma_start(wqk_sb[:, half * 512:(half + 1) * 512],
                              wqk_d[:, half * 512:(half + 1) * 512])
        for a in range(NCH):
            sl = slice(a * NF + 512, a * NF + NF)
            nc.sync.dma_start(cs_sb[:, sl], cs_d[:, sl])
            nc.sync.dma_start(sn_sb[:, sl], sn_d[:, sl])

        wv_sb = pool.tile([128, H * E], f16, tag="wv")
        nc.sync.dma_start(wv_sb[:], wv_d[:])
        wo_sb = pool.tile([128, H * E], f16, tag="wo")
        nc.sync.dma_start(wo_sb[:], wo_d[:])
        mhalf_sb = pool.tile([1, 1], f16, tag="mhalf")
        nc.sync.dma_start(mhalf_sb[:], mhalf_d[:])
        mhrow_sb = pool.tile([1, NF], f16, tag="mhrow")
        nc.sync.dma_start(mhrow_sb[:], mhrow_d[:])
        id_sb = pool.tile([128, 128], f32r, tag="ident")
        nc.sync.dma_start(id_sb[:], id_d[:])
        bv_sb = pool.tile([128, H], f32, tag="bv")
        nc.sync.dma_start(bv_sb[:], bv_d[:])
        bo_sb = pool.tile([128, 1], f32, tag="bo")
        nc.sync.dma_start(bo_sb[:], bo_d[:])

        # iota prep (gpsimd + vector idle at start)
        negpi = pool.tile([128, 1], f32, tag="negpi")
        nc.gpsimd.memset(negpi[:], float(-np.pi))
        b512 = pool.tile([128, 1], f32, tag="b512c")
        nc.gpsimd.memset(b512[:], 512.0)
        # throughput probe: measure gpsimd [128,512] f32->f16 copy in the trace
        gp_probe = pool.tile([128, 512], f16, tag="gprobe")
        nc.gpsimd.tensor_copy(gp_probe[:], xc[:].bitcast(f32))
        io_i = pool.tile([128, NF], i32, tag="ioi")
        nc.gpsimd.iota(io_i[:], pattern=[[1, NF]], base=0, channel_multiplier=0)
        io_f = pool.tile([128, NF], f32, tag="iof")      # 0..1023 per partition
        nc.vector.tensor_copy(io_f[:], io_i[:])

        # ---------------- forward DFT + QK projections, interleaved per freq-half ----------------
        hre = pool.tile([128, NF], f16, tag="hre")
        him = pool.tile([128, NF], f16, tag="him")
        hn = pool.tile([128, 1], f16, tag="hn")
        qk_t = {}
        for h in range(H):
            for c in range(2):
                qk_t[(h, c)] = (pool.tile([128, NF], f16, tag=f"q{c}h{h}", name=f"q{c}h{h}"),
                                pool.tile([128, NF], f16, tag=f"k{c}h{h}", name=f"k{c}h{h}"))
        pre_t = pool.tile([128, NCH * E], f16, tag="pre")
        pim_t = pool.tile([128, NCH * E], f16, tag="pim")
        t1 = pool.tile([128, 512], f16, tag="t1")
        t2 = pool.tile([128, 512], f16, tag="t2")
        t3 = pool.tile([128, 512], f16, tag="t3")
        t4 = pool.tile([128, 512], f16, tag="t4")

        for g in range(2):
            gsl = slice(g * 512, (g + 1) * 512)
            hre_ps = pb.tile([128, 512], f32, tag="b512")
            for a in range(NCH):
                nc.tensor.matmul(hre_ps[:], xc[:, a * E:(a + 1) * E],
                                 cs_sb[:, a * NF + g * 512: a * NF + (g + 1) * 512],
                                 start=(a == 0), stop=False)
            nc.tensor.matmul(hre_ps[:], xnyq[:], altf_sb[:, gsl], start=False, stop=True)
            nc.vector.tensor_copy(hre[:, gsl], hre_ps[:])
            him_ps = pb.tile([128, 512], f32, tag="b512")
            for a in range(NCH):
                nc.tensor.matmul(him_ps[:], xs[:, a * E:(a + 1) * E],
                                 sn_sb[:, a * NF + g * 512: a * NF + (g + 1) * 512],
                                 start=(a == 0), stop=(a == NCH - 1))
            nc.vector.tensor_copy(him[:, gsl], him_ps[:])
            if g == 0:
                hn_ps = prow.tile([128, 1], f32, tag="row")
                for a in range(NCH):
                    nc.tensor.matmul(hn_ps[:], xc[:, a * E:(a + 1) * E], altp_sb[:],
                                     start=(a == 0), stop=False)
                nc.tensor.matmul(hn_ps[:], xnyq[:], one_sb[:], start=False, stop=True)
                nc.vector.tensor_copy(hn[:], hn_ps[:])
            for c in range(2):
                hsrc = hre if c == 0 else him
                for h in range(H):
                    ps = pqk.tile([128, 1024], f32, tag="qk1024")
                    for jj in range(4):
                        j = g * 4 + jj
                        nc.tensor.matmul(ps[:, jj * 256:(jj + 1) * 256],
                                         hsrc[:, j * 128:(j + 1) * 128],
                                         wqk_sb[:, h * 256:(h + 1) * 256],
                                         start=True, stop=True)
                    if c == 0 and g == 0:
                        # DC-bin bias: q and k rows adjacent in interleaved bqk
                        nc.vector.tensor_add(ps[0:1, 0:256], ps[0:1, 0:256],
                                             bqk_sb[0:1, h * 256:(h + 1) * 256])
                    v3 = ps[:].rearrange("p (jj k e) -> k p jj e", jj=4, k=2)
                    qt, kt = qk_t[(h, c)]
                    r2 = lambda ap: ap.rearrange("p (jj e) -> p jj e", jj=4)
                    nc.scalar.copy(r2(qt[:, gsl]), v3[0])
                    nc.vector.tensor_copy(r2(kt[:, gsl]), v3[1])
            # products + accumulation for this freq-half
            for h in range(H):
                (qr, kr), (qi, ki) = qk_t[(h, 0)], qk_t[(h, 1)]
                nc.vector.tensor_tensor(t1[:], qr[:, gsl], kr[:, gsl], AL.mult)
                nc.vector.tensor_tensor(t2[:], qi[:, gsl], ki[:, gsl], AL.mult)
                nc.vector.tensor_tensor(t3[:], qi[:, gsl], kr[:, gsl], AL.mult)
                nc.vector.tensor_tensor(t4[:], qr[:, gsl], ki[:, gsl], AL.mult)
                if h == 0:
                    nc.vector.tensor_add(pre_t[:, gsl], t1[:], t2[:])
                    nc.vector.tensor_sub(pim_t[:, gsl], t3[:], t4[:])
                else:
                    nc.vector.tensor_add(pre_t[:, gsl], pre_t[:, gsl], t1[:])
                    nc.vector.tensor_add(pre_t[:, gsl], pre_t[:, gsl], t2[:])
                    nc.vector.tensor_add(pim_t[:, gsl], pim_t[:, gsl], t3[:])
                    nc.vector.tensor_sub(pim_t[:, gsl], pim_t[:, gsl], t4[:])

        # Nyquist rows per head (off the products critical path)
        qn_row = pool.tile([1, E], f32, tag="qnrow")
        kn_row = pool.tile([1, E], f32, tag="knrow")
        pn_row = pool.tile([1, E], f32, tag="pnrow")
        pn_f16 = pool.tile([1, E], f16, tag="pnf16")
        pnw = pool.tile([1, E], f32, tag="pnw")
        for h in range(H):
            r_ps = prow.tile([1, 256], f32, tag="row")
            nc.tensor.matmul(r_ps[:], hn[:], wqk_sb[:, h * 256:(h + 1) * 256],
                             start=True, stop=True)
            nc.scalar.copy(qn_row[:], r_ps[:, 0:128])
            nc.scalar.copy(kn_row[:], r_ps[:, 128:256])
            if h == 0:
                nc.vector.scalar_tensor_tensor(pn_row[:], qn_row[:], 0.5, kn_row[:], AL.mult, AL.mult)
            else:
                nc.vector.scalar_tensor_tensor(pnw[:], qn_row[:], 0.5, kn_row[:], AL.mult, AL.mult)
                nc.vector.tensor_add(pn_row[:], pn_row[:], pnw[:])
        nc.vector.tensor_copy(pn_f16[:], pn_row[:])

        # ---------------- acm inverse (single pass, unscaled 4x; fixed at softmax) ----------------
        eo_sb = pool.tile([128, T], f32, tag="eo")
        e_acm = pqk.tile([128, 1024], f32, tag="qk1024")
        o_acm = pqk.tile([128, 1024], f32, tag="qk1024")
        # acm[t=1024] first: small serial chain, off the top-k critical path
        a1024_ps = prow.tile([128, 1], f32, tag="row")
        for j in range(NCH):
            nc.tensor.matmul(a1024_ps[:], pre_t[:, j * E:(j + 1) * E], altp_sb[:],
                             start=(j == 0), stop=False)
        nc.tensor.matmul(a1024_ps[:], pn_f16[:], one_sb[:], start=False, stop=True)
        nc.scalar.copy(eo_sb[:, NF:NF + 1], a1024_ps[:])
        for s in range(2):
            sl = slice(s * 512, (s + 1) * 512)
            for j in range(NCH):
                nc.tensor.matmul(e_acm[:, sl], pre_t[:, j * E:(j + 1) * E],
                                 cs_sb[:, j * NF + s * 512: j * NF + (s + 1) * 512],
                                 start=(j == 0), stop=False)
            nc.tensor.matmul(e_acm[:, sl], pn_f16[:], altf_sb[:, sl], start=False, stop=True)
            for j in range(NCH):
                nc.tensor.matmul(o_acm[:, sl], pim_t[:, j * E:(j + 1) * E],
                                 sn_sb[:, j * NF + s * 512: j * NF + (s + 1) * 512],
                                 start=(j == 0), stop=(j == NCH - 1))
            e_sb = pool2.tile([128, 512], f32, tag="ecp")
            nc.scalar.copy(e_sb[:], e_acm[:, sl])
            nc.vector.tensor_sub(eo_sb[:, sl], e_sb[:], o_acm[:, sl])
            if s == 0:
                nc.vector.tensor_add(eo_sb[:, NF + 1:NF + 512], e_sb[:, 1:], o_acm[:, sl][:, 1:])
            else:
                nc.vector.tensor_add(eo_sb[:, NF + 512:2 * NF], e_sb[:], o_acm[:, sl])

        # ---------------- V spectra (fills PE/scalar idle under top-k) ----------------
        vn_cols = pool.tile([128, H], f32, tag="vncols")
        vre_t, vim_t = {}, {}
        for h in range(H):
            for c in range(2):
                hsrc = hre if c == 0 else him
                vt = pool.tile([128, NF], f16, tag=f"v{c}h{h}")
                (vre_t if c == 0 else vim_t)[h] = vt
                for sv in range(2):
                    v_ps = pb.tile([128, 512], f32, tag="b512")
                    nc.tensor.matmul(v_ps[:], wv_sb[:, h * E:(h + 1) * E],
                                     hsrc[:, sv * 512:(sv + 1) * 512], start=True, stop=True)
                    if c == 0 and sv == 0:
                        nc.vector.tensor_add(v_ps[:, 0:1], v_ps[:, 0:1], bv_sb[:, h:h + 1])
                    # scale 1/64 keeps wo_a = wo*(wgt/16) in fp16 normal range
                    nc.scalar.activation(vt[:, sv * 512:(sv + 1) * 512], v_ps[:],
                                         AF.Identity, scale=1.0 / 64.0)
            vn_ps = prow.tile([128, 1], f32, tag="row")
            nc.tensor.matmul(vn_ps[:], wv_sb[:, h * E:(h + 1) * E], hn[:], start=True, stop=True)
            nc.scalar.copy(vn_cols[:, h:h + 1], vn_ps[:])

        # ---------------- top-k ----------------
        vals = pool.tile([128, 24], f32, tag="vals")
        expv = pool.tile([128, 24], f32, tag="expv")
        negv0 = pool.tile([128, 1], f32, tag="negv0")
        nc.vector.max(vals[:, 0:8], eo_sb[:])
        nc.vector.tensor_scalar_mul(negv0[:], vals[:, 0:1], -0.25)
        nc.vector.match_replace(eo_sb[:], vals[:, 0:8], eo_sb[:], -1e30)
        nc.vector.max(vals[:, 8:16], eo_sb[:])
        # softmax exp for first 16 values issues while round 3 runs
        nc.scalar.activation(expv[:, 0:16], vals[:, 0:16], AF.Exp, bias=negv0[:], scale=0.25)
        den1 = pool.tile([128, 1], f32, tag="den1")
        nc.vector.tensor_reduce(den1[:], expv[:, 0:16], AX.X, AL.add)
        nc.vector.match_replace(eo_sb[:], vals[:, 8:16], eo_sb[:], -1e30)
        nc.vector.max(vals[:, 16:24], eo_sb[:])
        nc.scalar.activation(expv[:, 16:24], vals[:, 16:24], AF.Exp, bias=negv0[:], scale=0.25)
        # preload Sin activation table behind a dummy op (after last Exp)
        sin_dummy = pool.tile([1, 1], f32, tag="sdum")
        nc.scalar.activation(sin_dummy[:], one_sb[:], AF.Sin, scale=0.1)

        # delay via index search (A/B: v2-style max_index + arithmetic remap)
        idx8 = pool.tile([128, 8], u32, tag="idx8")
        nc.vector.max_index(idx8[:], vals[:, 16:24], eo_sb[:])
        c_i = pool.tile([128, 1], i32, tag="ci")
        nc.vector.tensor_copy(c_i[:], idx8[:, 5:6].bitcast(i32))
        c_neg = pool.tile([128, 1], i32, tag="cneg")
        nc.vector.tensor_scalar(c_neg[:], c_i[:], -1, 3072, AL.mult, AL.add)
        mask = pool.tile([128, 1], i32, tag="mask")
        nc.vector.tensor_scalar(mask[:], c_i[:], 1024, None, AL.is_gt)
        d_sel = pool.tile([128, 1], i32, tag="dsel")
        nc.vector.select(d_sel[:], mask[:], c_neg[:], c_i[:])
        d_f = pool.tile([128, 1], f32, tag="df")
        nc.vector.tensor_copy(d_f[:], d_sel[:])

        # ---------------- phases (sw = phs[0], cw = phs[1]; wgt folded into wo_a) ----------------
        # sw-arg = (j*d) mod 2048 ; cw-arg = (sw-arg + 512) mod 2048 (int chain;
        # arith ops cast on write, bitVec must keep dtype)
        mi0 = pool2.tile([128, NF], i32, tag="pmi")
        mi1 = pool2.tile([128, NF], i32, tag="pmi")
        mf0 = pool2.tile([128, NF], f16, tag="pmf")   # args <= 2047: exact in fp16
        mf1 = pool2.tile([128, NF], f16, tag="pmf")
        # j*d (+off) on the scalar engine (exact ints < 2^21), mod+downcast on vector
        nc.scalar.activation(mi0[:], io_f[:], AF.Identity, scale=d_f[:])
        nc.scalar.activation(mi1[:], io_f[:], AF.Identity, scale=d_f[:], bias=b512[:])
        nc.vector.tensor_scalar(mi0[:], mi0[:], 2047, None, AL.bitwise_and)
        nc.vector.tensor_copy(mf0[:], mi0[:])
        nc.vector.tensor_scalar(mi1[:], mi1[:], 2047, None, AL.bitwise_and)
        nc.vector.tensor_copy(mf1[:], mi1[:])
        nc.scalar.activation(mf0[:], mf0[:], AF.Sin,
                             scale=float(np.pi / 1024.0), bias=negpi[:])
        nc.scalar.activation(mf1[:], mf1[:], AF.Sin,
                             scale=float(np.pi / 1024.0), bias=negpi[:])
        phs = [mf0[:], mf1[:]]
        # softmax weight; fold (wgt * -2/T) into wo -> wo_a / won_a
        den = pool.tile([128, 1], f32, tag="den")
        nc.vector.tensor_reduce(den[:], expv[:, 16:22], AX.X, AL.add)
        nc.vector.tensor_add(den[:], den[:], den1[:])
        rden = pool.tile([128, 1], f32, tag="rden")
        nc.vector.reciprocal(rden[:], den[:])
        wgt = pool.tile([128, 1], f32, tag="wgt")
        nc.vector.tensor_mul(wgt[:], expv[:, 21:22], rden[:])
        alpha = pool.tile([128, 1], f32, tag="alpha")
        nc.vector.tensor_scalar_mul(alpha[:], wgt[:], -2.0 / T * 64.0)
        wo_a = pool.tile([128, H * E], f16, tag="woa")
        won_a = pool.tile([128, H * E], f16, tag="wona")
        nc.vector.tensor_scalar(wo_a[:], wo_sb[:], alpha[:], None, AL.mult)
        nc.vector.tensor_scalar(won_a[:], wo_a[:], -1.0, None, AL.mult)
        sw, cw = phs[0], phs[1]
        # nyquist scale: (1-2*(d&1)) * wgt / T
        d_i = pool.tile([128, 1], i32, tag="di")
        nc.vector.tensor_copy(d_i[:], d_f[:])
        par_i = pool.tile([128, 1], i32, tag="par")
        nc.vector.tensor_scalar(par_i[:], d_i[:], 1, None, AL.bitwise_and)
        parf = pool.tile([128, 1], f32, tag="parf")
        nc.vector.tensor_copy(parf[:], par_i[:])
        nc.vector.tensor_scalar(parf[:], parf[:], -2.0, 1.0, AL.mult, AL.add)
        nys = pool.tile([128, 1], f32, tag="nys")
        nc.vector.tensor_scalar(nys[:], parf[:], wgt[:], 1.0 / T, AL.mult, AL.mult)

        # ---------------- phase multiply + output projection (wo_a stationary) ----------------
        gn_cols = pool.tile([128, H], f16, tag="gncols")
        og_re = pqk.tile([128, 1024], f32, tag="qk1024")
        og_im = pqk.tile([128, 1024], f32, tag="qk1024")
        for h in range(H):
            vre, vim = vre_t[h], vim_t[h]
            nc.vector.tensor_scalar(gn_cols[:, h:h + 1], vn_cols[:, h:h + 1], nys[:], None, AL.mult)
            m1 = pool2.tile([128, NF], f16, tag="m1")
            m2 = pool2.tile([128, NF], f16, tag="m2")
            m3 = pool2.tile([128, NF], f16, tag="m3")
            m4 = pool2.tile([128, NF], f16, tag="m4")
            nc.vector.tensor_tensor(m1[:], vre[:], cw, AL.mult)
            nc.vector.tensor_tensor(m2[:], vim[:], sw, AL.mult)
            nc.vector.tensor_tensor(m3[:], vre[:], sw, AL.mult)
            nc.vector.tensor_tensor(m4[:], vim[:], cw, AL.mult)
            for s in range(2):
                sl = slice(s * 512, (s + 1) * 512)
                nc.tensor.matmul(og_re[:, sl], wo_a[:, h * E:(h + 1) * E], m1[:, sl],
                                 start=(h == 0), stop=False)
                nc.tensor.matmul(og_re[:, sl], won_a[:, h * E:(h + 1) * E], m2[:, sl],
                                 start=False, stop=(h == H - 1))
                nc.tensor.matmul(og_im[:, sl], wo_a[:, h * E:(h + 1) * E], m3[:, sl],
                                 start=(h == 0), stop=False)
                nc.tensor.matmul(og_im[:, sl], wo_a[:, h * E:(h + 1) * E], m4[:, sl],
                                 start=False, stop=(h == H - 1))
        ofn_ps = prow.tile([1, E], f32, tag="row")
        for h in range(H):
            nc.tensor.matmul(ofn_ps[:], gn_cols[:, h:h + 1], wo_sb[:, h * E:(h + 1) * E],
                             start=(h == 0), stop=(h == H - 1))
        ofn_row = pool.tile([1, E], f16, tag="ofnrow")
        nc.vector.tensor_copy(ofn_row[:], ofn_ps[:])

        # transpose og [e'', f] -> of [f, e''] via PE (f32r); og->g2 on vector (idle)
        g2_re = pool.tile([128, 1024], f32r, tag="g2re")
        g2_im = pool.tile([128, 1024], f32r, tag="g2im")
        nc.vector.tensor_copy(g2_re[:], og_re[:])
        nc.vector.tensor_copy(g2_im[:], og_im[:])
        of_re = pool.tile([128, NCH * E], f16, tag="ofre")
        of_im = pool.tile([128, NCH * E], f16, tag="ofim")
        id_r = id_sb[:]
        for half in range(4):
            tp = pb.tile([128, 512], f32, tag="b512")
            src = g2_re if half < 2 else g2_im
            dst = of_re if half < 2 else of_im
            base = (half % 2) * 512
            for q in range(4):
                j = (half % 2) * 4 + q
                nc.tensor.transpose(tp[:, q * 128:(q + 1) * 128].bitcast(f32r),
                                    src[:, j * 128:(j + 1) * 128], id_r)
            nc.scalar.copy(dst[:, base:base + 512], tp[:])

        # t = 1024 output row first so its DMA never gates the teardown
        o1_ps = prow.tile([128, 1], f32, tag="row")
        for j in range(NCH):
            nc.tensor.matmul(o1_ps[:], of_re[:, j * E:(j + 1) * E], altp_sb[:],
                             start=(j == 0), stop=False)
        nc.tensor.matmul(o1_ps[:], ofn_row[:], one_sb[:], start=False, stop=False)
        nc.tensor.matmul(o1_ps[:], of_re[0:1, 0:E], mhalf_sb[:], start=False, stop=True)
        o1_sb = pool.tile([128, 1], f32, tag="o1sb")
        nc.vector.tensor_scalar(o1_sb[:], o1_ps[:], bo_sb[:], None, AL.add)
        nc.sync.dma_start(o1024_d[:], o1_sb[:])

        # final inverse per s-half: op first (feeds scalar), then ep; combine via
        # scalar (op+bo / -op+bo straight from PSUM) + vector add of ep
        for s in range(2):
            sl = slice(s * 512, (s + 1) * 512)
            op_ps = pb.tile([128, 512], f32, tag="b512")
            ep_ps = pb.tile([128, 512], f32, tag="b512")
            for j in range(NCH):
                nc.tensor.matmul(op_ps[:], of_im[:, j * E:(j + 1) * E],
                                 sn_sb[:, j * NF + s * 512: j * NF + (s + 1) * 512],
                                 start=(j == 0), stop=(j == NCH - 1))
            for j in range(NCH):
                nc.tensor.matmul(ep_ps[:], of_re[:, j * E:(j + 1) * E],
                                 cs_sb[:, j * NF + s * 512: j * NF + (s + 1) * 512],
                                 start=(j == 0), stop=False)
            nc.tensor.matmul(ep_ps[:], ofn_row[:], altf_sb[:, sl], start=False, stop=False)
            nc.tensor.matmul(ep_ps[:], of_re[0:1, 0:E], mhrow_sb[:, sl], start=False, stop=True)
            tpos = pool2.tile([128, 512], f32, tag="tpos", bufs=1)
            tneg = pool2.tile([128, 512], f32, tag="tneg", bufs=1)
            nc.scalar.activation(tpos[:], op_ps[:], AF.Identity, bias=bo_sb[:], scale=1.0)
            nc.scalar.activation(tneg[:], op_ps[:], AF.Identity, bias=bo_sb[:], scale=-1.0)
            out_lo = pool2.tile([128, 512], f32, tag="outlo")
            out_hi = pool2.tile([128, 512], f32, tag="outhi")
            nc.vector.tensor_add(out_lo[:], ep_ps[:], tneg[:])
            nc.vector.tensor_add(out_hi[:], ep_ps[:], tpos[:])
            nc.sync.dma_start(lo_d[:, sl], out_lo[:])
            nc.sync.dma_start(hi_d[:, sl], out_hi[:])

    nc.compile()
    return nc


def _get_nc():
    if "nc" not in _CACHE:
        _wire_ntff_hook()
        _CACHE["nc"] = _build()
    return _CACHE["nc"]


def kernel(hidden_states, wq, bq, wk, bk, wv, bv, wo, bo):
    global LAST_EXEC_NS
    nc = _get_nc()
    consts = _CACHE.setdefault("consts", _host_consts())

    def chunked(a):
        # [1024, W] -> [128, 8*W] with chunk-major columns (device layout)
        W = a.shape[1]
        return np.ascontiguousarray(
            a.reshape(NCH, 128, W).transpose(1, 0, 2).reshape(128, NCH * W))

    hs = np.ascontiguousarray(hidden_states, dtype=np.float32)
    wqk = np.ascontiguousarray(
        (np.concatenate([wq.transpose(2, 0, 1), wk.transpose(2, 0, 1)], axis=2)
         * (1.0 / 32.0)).transpose(1, 0, 2).reshape(128, H * 256)).astype(np.float16)
    wv_h = np.ascontiguousarray(
        wv.transpose(2, 0, 1).transpose(1, 0, 2).reshape(128, H * E)).astype(np.float16)
    wo_h = np.ascontiguousarray(
        wo.transpose(1, 0, 2).transpose(1, 0, 2).reshape(128, H * E)).astype(np.float16)
    # interleaved per head: [q_h (128) | k_h (128)] * 4, scaled (T/32)
    bqk = np.ascontiguousarray(
        np.stack([(T / 32.0) * bq.T, (T / 32.0) * bk.T], axis=1).reshape(-1)[None, :]
    ).astype(np.float32)                                                     # [1, 2*H*E]
    bv_s = np.ascontiguousarray(T * bv, dtype=np.float32)                     # [E, H]
    bo_c = np.ascontiguousarray(bo, dtype=np.float32)[:, None]                # [E, 1]

    in_maps = []
    for b in range(B):
        x = hs[b]
        xr = np.concatenate([x[0:1], x[:0:-1]])[:NF]
        xc = (x[:NF] + xr)
        xc[0] *= 0.5
        xs = (xr - x[:NF])
        in_maps.append({
            "xc": chunked(xc).astype(np.float16), "xs": chunked(xs).astype(np.float16),
            "xnyq": x[NF:NF + 1].astype(np.float16),
            "cs": consts["cs"], "sn": consts["sn"], "altf": consts["altf"],
            "altp": consts["altp"], "one": consts["one"], "mhalf": consts["mhalf"],
            "mhrow": consts["mhrow"], "ident": consts["ident"],
            "wqk": wqk, "wv": wv_h, "wo": wo_h, "bqk": bqk, "bv": bv_s, "bo": bo_c,
        })

    trace = bool(int(os.environ.get("BASS_KERNEL_TRACE", "0")))
    res = run_bass_kernel_spmd(nc, in_maps, core_ids=list(range(B)), trace=trace)
    LAST_EXEC_NS = res.exec_time_ns
    _CACHE["last_res"] = res

    out = np.empty((B, T, E), dtype=np.float32)
    for b in range(B):
        r = res.results[b]
        out[b, 0:NF] = r["out_lo"].T
        out[b, NF] = r["out_1024"][:, 0]
        out[b, NF + 1:] = r["out_hi"][:, 1:NF][:, ::-1].T
    return out


# revision 36
# speedup vs baseline: 1.0942x; 1.0942x over previous
"""AutoCorrelation (Autoformer-style) Trainium2 Bass kernel, v3.

Sharding: data-parallel over batch — 8 batch elements -> 8 NeuronCores, no
collectives. Each core computes its [2048, 128] output slice independently.

Same math as v2; restructured schedule:
  * consumption-ordered per-chunk DMA of the cos/sin DFT tables so the
    forward DFT streams behind the DMA instead of waiting for the full 4MB;
  * QK projections per (freq-half, comp, head); all PSUM evacuations on the
    scalar engine; complex products split per freq-half so the vector engine
    starts ~6us earlier; per-head dedicated SBUF tiles (no pool rotation);
  * single combined acm inverse (all 4 heads accumulated into one pre/pim);
  * V spectra issued after the inverse so they fill the PE/scalar idle
    window under the top-k scans;
  * top-k tail: delay index via fused (eo == v21) * remap_table accumulate
    (one DVE op instead of FIND_INDEX8 + int chain); phase args via two
    dtype-converting tensor_scalar ops (f32 mult -> i32, and -> f32);
    Sin activation table preloaded behind a dummy op; softmax weight and
    -2/T folded into the output projection weights (wo_a / won_a);
  * final combine: op+bo / -op+bo on scalar directly from PSUM, vector adds
    ep; og->f32r copies on the (then idle) vector engine.
"""
import os
import sys
import types
from contextlib import ExitStack

sys.path.insert(0, "/opt/trn_rl_repo")

import numpy as np

import concourse.bass as bass
import concourse.mybir as mybir
from concourse import bacc
from concourse.tile import TileContext
from concourse.bass_utils import run_bass_kernel_spmd

B, T, E, H = 8, 2048, 128, 4
NF = 1024
NCH = 8
AL = mybir.AluOpType
DT = mybir.dt
AF = mybir.ActivationFunctionType
AX = mybir.AxisListType

_CACHE = {}
LAST_EXEC_NS = None


def _wire_ntff_hook():
    if "antenv.axon_hooks" in sys.modules:
        return
    try:
        mod = types.ModuleType("antenv.axon_hooks")
        _h = [None]
        mod.set_axon_ntff_profile_hook = lambda h: _h.__setitem__(0, h)
        mod.get_axon_ntff_profile_hook = lambda: _h[0]
        sys.modules["antenv.axon_hooks"] = mod
        import antenv
        antenv.axon_hooks = mod
        from trn_agent_boot.trn_boot import _ntff_profile_via_ctypes
        mod.set_axon_ntff_profile_hook(_ntff_profile_via_ctypes("/opt/axon/libaxon_pjrt.so"))
    except Exception:
        pass


def _host_consts():
    i = np.arange(NF, dtype=np.float64)
    ang = np.outer(i, i) * (2.0 * np.pi / T)

    def chunk(a):  # [1024,1024] -> [128, 8*1024] chunk-major
        return np.ascontiguousarray(
            a.reshape(NCH, 128, NF).transpose(1, 0, 2).reshape(128, NCH * NF))

    return {
        "cs": chunk(np.cos(ang).astype(np.float16)),
        "sn": chunk(np.sin(ang).astype(np.float16)),
        "altf": ((-1.0) ** np.arange(NF)).astype(np.float16)[None, :],
        "altp": ((-1.0) ** np.arange(128)).astype(np.float16)[:, None],
        "one": np.ones((1, 1), np.float16),
        "mhalf": np.full((1, 1), -0.5, np.float16),
        "mhrow": np.full((1, NF), -0.5, np.float16),
        "ident": np.eye(128, dtype=np.float32),
    }


def _build():
    nc = bacc.Bacc("TRN2", target_bir_lowering=False, debug=False, num_devices=1)
    f32, f32r, f16, i32, u32 = DT.float32, DT.float32r, DT.float16, DT.int32, DT.uint32

    # all 2D tensors ship host-pre-chunked: [partition, chunk-major free]
    xc_d = nc.dram_tensor("xc", [128, NCH * E], f16, kind="ExternalInput")
    xs_d = nc.dram_tensor("xs", [128, NCH * E], f16, kind="ExternalInput")
    xnyq_d = nc.dram_tensor("xnyq", [1, E], f16, kind="ExternalInput")
    cs_d = nc.dram_tensor("cs", [128, NCH * NF], f16, kind="ExternalInput")
    sn_d = nc.dram_tensor("sn", [128, NCH * NF], f16, kind="ExternalInput")
    altf_d = nc.dram_tensor("altf", [1, NF], f16, kind="ExternalInput")
    altp_d = nc.dram_tensor("altp", [128, 1], f16, kind="ExternalInput")
    one_d = nc.dram_tensor("one", [1, 1], f16, kind="ExternalInput")
    mhalf_d = nc.dram_tensor("mhalf", [1, 1], f16, kind="ExternalInput")
    mhrow_d = nc.dram_tensor("mhrow", [1, NF], f16, kind="ExternalInput")
    id_d = nc.dram_tensor("ident", [128, 128], f32r, kind="ExternalInput")
    wqk_d = nc.dram_tensor("wqk", [128, H * 256], f16, kind="ExternalInput")  # pre-scaled 1/32
    wv_d = nc.dram_tensor("wv", [128, H * E], f16, kind="ExternalInput")
    wo_d = nc.dram_tensor("wo", [128, H * E], f16, kind="ExternalInput")
    bqk_d = nc.dram_tensor("bqk", [1, 2 * H * E], f32, kind="ExternalInput")  # interleaved (q_h|k_h)*4
    bv_d = nc.dram_tensor("bv", [E, H], f32, kind="ExternalInput")        # T*bv
    bo_d = nc.dram_tensor("bo", [E, 1], f32, kind="ExternalInput")
    lo_d = nc.dram_tensor("out_lo", [E, NF], f32, kind="ExternalOutput")
    hi_d = nc.dram_tensor("out_hi", [E, NF], f32, kind="ExternalOutput")
    o1024_d = nc.dram_tensor("out_1024", [E, 1], f32, kind="ExternalOutput")

    with TileContext(nc) as tc, ExitStack() as ctx:
        pool = ctx.enter_context(tc.tile_pool(name="main", bufs=1))
        pool2 = ctx.enter_context(tc.tile_pool(name="rot", bufs=2))
        pqk = ctx.enter_context(tc.tile_pool(name="pqk", bufs=2, space="PSUM"))    # [128,1024] tiles
        pb = ctx.enter_context(tc.tile_pool(name="pb", bufs=3, space="PSUM"))      # [128,512] tiles
        prow = ctx.enter_context(tc.tile_pool(name="psrow", bufs=1, space="PSUM"))

        # ---------------- loads, consumption-ordered, split for queue parallelism ----------------
        xnyq = pool.tile([1, E], f16, tag="xnyq")
        nc.sync.dma_start(xnyq[:], xnyq_d[:])
        altf_sb = pool.tile([1, NF], f16, tag="altf")
        nc.sync.dma_start(altf_sb[:], altf_d[:])
        altp_sb = pool.tile([128, 1], f16, tag="altp")
        nc.sync.dma_start(altp_sb[:], altp_d[:])
        one_sb = pool.tile([1, 1], f16, tag="one")
        nc.sync.dma_start(one_sb[:], one_d[:])
        bqk_sb = pool.tile([1, 2 * H * E], f32, tag="bqk")
        nc.sync.dma_start(bqk_sb[:], bqk_d[:])
        xc = pool.tile([128, NCH * E], f16, tag="xc")
        xs = pool.tile([128, NCH * E], f16, tag="xs")
        cs_sb = pool.tile([128, NCH * NF], f16, tag="cs")
        sn_sb = pool.tile([128, NCH * NF], f16, tag="sn")
        for half in range(2):
            nc.sync.dma_start(xc[:, half * 512:(half + 1) * 512],
                              xc_d[:, half * 512:(half + 1) * 512])
        for half in range(2):
            nc.sync.dma_start(xs[:, half * 512:(half + 1) * 512],
                              xs_d[:, half * 512:(half + 1) * 512])
        # s0 table pieces interleaved cs/sn so hre/him pace together
        for a in range(NCH):
            sl = slice(a * NF, a * NF + 512)
            nc.sync.dma_start(cs_sb[:, sl], cs_d[:, sl])
            nc.sync.dma_start(sn_sb[:, sl], sn_d[:, sl])
        wqk_sb = pool.tile([128, H * 256], f16, tag="wqk")
        for half in range(2):
            nc.sync.dma_start(wqk_sb[:, half * 512:(half + 1) * 512],
                              wqk_d[:, half * 512:(half + 1) * 512])
        for a in range(NCH):
            sl = slice(a * NF + 512, a * NF + NF)
            nc.sync.dma_start(cs_sb[:, sl], cs_d[:, sl])
            nc.sync.dma_start(sn_sb[:, sl], sn_d[:, sl])

        wv_sb = pool.tile([128, H * E], f16, tag="wv")
        nc.sync.dma_start(wv_sb[:], wv_d[:])
        wo_sb = pool.tile([128, H * E], f16, tag="wo")
        nc.sync.dma_start(wo_sb[:], wo_d[:])
        mhalf_sb = pool.tile([1, 1], f16, tag="mhalf")
        nc.sync.dma_start(mhalf_sb[:], mhalf_d[:])
        mhrow_sb = pool.tile([1, NF], f16, tag="mhrow")
        nc.sync.dma_start(mhrow_sb[:], mhrow_d[:])
        id_sb = pool.tile([128, 128], f32r, tag="ident")
        nc.sync.dma_start(id_sb[:], id_d[:])
        bv_sb = pool.tile([128, H], f32, tag="bv")
        nc.sync.dma_start(bv_sb[:], bv_d[:])
        bo_sb = pool.tile([128, 1], f32, tag="bo")
        nc.sync.dma_start(bo_sb[:], bo_d[:])

        # iota prep (gpsimd + vector idle at start)
        negpi = pool.tile([128, 1], f32, tag="negpi")
        nc.gpsimd.memset(negpi[:], float(-np.pi))
        b512 = pool.tile([128, 1], f32, tag="b512c")
        nc.gpsimd.memset(b512[:], 512.0)

        io_i = pool.tile([128, NF], i32, tag="ioi")
        nc.gpsimd.iota(io_i[:], pattern=[[1, NF]], base=0, channel_multiplier=0)
        io_f = pool.tile([128, NF], f32, tag="iof")      # 0..1023 per partition
        nc.vector.tensor_copy(io_f[:], io_i[:])

        # ---------------- forward DFT + QK projections, interleaved per freq-half ----------------
        hre = pool.tile([128, NF], f16, tag="hre")
        him = pool.tile([128, NF], f16, tag="him")
        hn = pool.tile([128, 1], f16, tag="hn")
        qk_t = {}
        for h in range(H):
            for c in range(2):
                qk_t[(h, c)] = (pool.tile([128, NF], f16, tag=f"q{c}h{h}", name=f"q{c}h{h}"),
                                pool.tile([128, NF], f16, tag=f"k{c}h{h}", name=f"k{c}h{h}"))
        pre_t = pool.tile([128, NCH * E], f16, tag="pre")
        pim_t = pool.tile([128, NCH * E], f16, tag="pim")
        t1 = pool.tile([128, 512], f16, tag="t1")
        t2 = pool.tile([128, 512], f16, tag="t2")
        t3 = pool.tile([128, 512], f16, tag="t3")
        t4 = pool.tile([128, 512], f16, tag="t4")

        for g in range(2):
            gsl = slice(g * 512, (g + 1) * 512)
            hre_ps = pb.tile([128, 512], f32, tag="b512")
            for a in range(NCH):
                nc.tensor.matmul(hre_ps[:], xc[:, a * E:(a + 1) * E],
                                 cs_sb[:, a * NF + g * 512: a * NF + (g + 1) * 512],
                                 start=(a == 0), stop=False)
            nc.tensor.matmul(hre_ps[:], xnyq[:], altf_sb[:, gsl], start=False, stop=True)
            nc.vector.tensor_copy(hre[:, gsl], hre_ps[:])
            him_ps = pb.tile([128, 512], f32, tag="b512")
            for a in range(NCH):
                nc.tensor.matmul(him_ps[:], xs[:, a * E:(a + 1) * E],
                                 sn_sb[:, a * NF + g * 512: a * NF + (g + 1) * 512],
                                 start=(a == 0), stop=(a == NCH - 1))
            nc.vector.tensor_copy(him[:, gsl], him_ps[:])
            if g == 0:
                hn_ps = prow.tile([128, 1], f32, tag="row")
                for a in range(NCH):
                    nc.tensor.matmul(hn_ps[:], xc[:, a * E:(a + 1) * E], altp_sb[:],
                                     start=(a == 0), stop=False)
                nc.tensor.matmul(hn_ps[:], xnyq[:], one_sb[:], start=False, stop=True)
                nc.vector.tensor_copy(hn[:], hn_ps[:])
            for h in range(H):
                for c in range(2):
                    hsrc = hre if c == 0 else him
                    ps = pqk.tile([128, 1024], f32, tag="qk1024")
                    for jj in range(4):
                        j = g * 4 + jj
                        nc.tensor.matmul(ps[:, jj * 256:(jj + 1) * 256],
                                         hsrc[:, j * 128:(j + 1) * 128],
                                         wqk_sb[:, h * 256:(h + 1) * 256],
                                         start=True, stop=True)
                    if c == 0 and g == 0:
                        # DC-bin bias: q and k rows adjacent in interleaved bqk
                        nc.vector.tensor_add(ps[0:1, 0:256], ps[0:1, 0:256],
                                             bqk_sb[0:1, h * 256:(h + 1) * 256])
                    v3 = ps[:].rearrange("p (jj k e) -> k p jj e", jj=4, k=2)
                    qt, kt = qk_t[(h, c)]
                    r2 = lambda ap: ap.rearrange("p (jj e) -> p jj e", jj=4)
                    nc.scalar.copy(r2(qt[:, gsl]), v3[0])
                    nc.scalar.copy(r2(kt[:, gsl]), v3[1])
                # products + accumulation for this head/freq-half
                (qr, kr), (qi, ki) = qk_t[(h, 0)], qk_t[(h, 1)]
                nc.vector.tensor_tensor(t1[:], qr[:, gsl], kr[:, gsl], AL.mult)
                nc.vector.tensor_tensor(t2[:], qi[:, gsl], ki[:, gsl], AL.mult)
                nc.vector.tensor_tensor(t3[:], qi[:, gsl], kr[:, gsl], AL.mult)
                nc.vector.tensor_tensor(t4[:], qr[:, gsl], ki[:, gsl], AL.mult)
                if h == 0:
                    nc.vector.tensor_add(pre_t[:, gsl], t1[:], t2[:])
                    nc.vector.tensor_sub(pim_t[:, gsl], t3[:], t4[:])
                else:
                    nc.vector.tensor_add(pre_t[:, gsl], pre_t[:, gsl], t1[:])
                    nc.vector.tensor_add(pre_t[:, gsl], pre_t[:, gsl], t2[:])
                    nc.vector.tensor_add(pim_t[:, gsl], pim_t[:, gsl], t3[:])
                    nc.vector.tensor_sub(pim_t[:, gsl], pim_t[:, gsl], t4[:])

        # Nyquist rows per head (off the products critical path)
        qn_row = pool.tile([1, E], f32, tag="qnrow")
        kn_row = pool.tile([1, E], f32, tag="knrow")
        pn_row = pool.tile([1, E], f32, tag="pnrow")
        pn_f16 = pool.tile([1, E], f16, tag="pnf16")
        pnw = pool.tile([1, E], f32, tag="pnw")
        for h in range(H):
            r_ps = prow.tile([1, 256], f32, tag="row")
            nc.tensor.matmul(r_ps[:], hn[:], wqk_sb[:, h * 256:(h + 1) * 256],
                             start=True, stop=True)
            nc.vector.tensor_copy(qn_row[:], r_ps[:, 0:128])
            nc.vector.tensor_copy(kn_row[:], r_ps[:, 128:256])
            if h == 0:
                nc.vector.scalar_tensor_tensor(pn_row[:], qn_row[:], 0.5, kn_row[:], AL.mult, AL.mult)
            else:
                nc.vector.scalar_tensor_tensor(pnw[:], qn_row[:], 0.5, kn_row[:], AL.mult, AL.mult)
                nc.vector.tensor_add(pn_row[:], pn_row[:], pnw[:])
        nc.vector.tensor_copy(pn_f16[:], pn_row[:])

        # ---------------- acm inverse (single pass, unscaled 4x; fixed at softmax) ----------------
        eo_sb = pool.tile([128, T], f32, tag="eo")
        e_acm = pqk.tile([128, 1024], f32, tag="qk1024")
        o_acm = pqk.tile([128, 1024], f32, tag="qk1024")
        # acm[t=1024] first: small serial chain, off the top-k critical path
        a1024_ps = prow.tile([128, 1], f32, tag="row")
        for j in range(NCH):
            nc.tensor.matmul(a1024_ps[:], pre_t[:, j * E:(j + 1) * E], altp_sb[:],
                             start=(j == 0), stop=False)
        nc.tensor.matmul(a1024_ps[:], pn_f16[:], one_sb[:], start=False, stop=True)
        nc.scalar.copy(eo_sb[:, NF:NF + 1], a1024_ps[:])
        for s in range(2):
            sl = slice(s * 512, (s + 1) * 512)
            for j in range(NCH):
                nc.tensor.matmul(e_acm[:, sl], pre_t[:, j * E:(j + 1) * E],
                                 cs_sb[:, j * NF + s * 512: j * NF + (s + 1) * 512],
                                 start=(j == 0), stop=False)
            nc.tensor.matmul(e_acm[:, sl], pn_f16[:], altf_sb[:, sl], start=False, stop=True)
            for j in range(NCH):
                nc.tensor.matmul(o_acm[:, sl], pim_t[:, j * E:(j + 1) * E],
                                 sn_sb[:, j * NF + s * 512: j * NF + (s + 1) * 512],
                                 start=(j == 0), stop=(j == NCH - 1))
            e_sb = pool2.tile([128, 512], f32, tag="ecp")
            nc.scalar.copy(e_sb[:], e_acm[:, sl])
            nc.vector.tensor_sub(eo_sb[:, sl], e_sb[:], o_acm[:, sl])
            if s == 0:
                nc.vector.tensor_add(eo_sb[:, NF + 1:NF + 512], e_sb[:, 1:], o_acm[:, sl][:, 1:])
            else:
                nc.vector.tensor_add(eo_sb[:, NF + 512:2 * NF], e_sb[:], o_acm[:, sl])

        # ---------------- V spectra (fills PE/scalar idle under top-k) ----------------
        vn_cols = pool.tile([128, H], f32, tag="vncols")
        vre_t, vim_t = {}, {}
        for h in range(H):
            for c in range(2):
                hsrc = hre if c == 0 else him
                vt = pool.tile([128, NF], f16, tag=f"v{c}h{h}")
                (vre_t if c == 0 else vim_t)[h] = vt
                for sv in range(2):
                    v_ps = pb.tile([128, 512], f32, tag="b512")
                    nc.tensor.matmul(v_ps[:], wv_sb[:, h * E:(h + 1) * E],
                                     hsrc[:, sv * 512:(sv + 1) * 512], start=True, stop=True)
                    if c == 0 and sv == 0:
                        nc.vector.tensor_add(v_ps[:, 0:1], v_ps[:, 0:1], bv_sb[:, h:h + 1])
                    # scale 1/64 keeps wo_a = wo*(wgt/16) in fp16 normal range
                    nc.scalar.activation(vt[:, sv * 512:(sv + 1) * 512], v_ps[:],
                                         AF.Identity, scale=1.0 / 64.0)
            vn_ps = prow.tile([128, 1], f32, tag="row")
            nc.tensor.matmul(vn_ps[:], wv_sb[:, h * E:(h + 1) * E], hn[:], start=True, stop=True)
            nc.scalar.copy(vn_cols[:, h:h + 1], vn_ps[:])

        # ---------------- top-k ----------------
        vals = pool.tile([128, 24], f32, tag="vals")
        expv = pool.tile([128, 24], f32, tag="expv")
        negv0 = pool.tile([128, 1], f32, tag="negv0")
        nc.vector.max(vals[:, 0:8], eo_sb[:])
        nc.vector.tensor_scalar_mul(negv0[:], vals[:, 0:1], -0.25)
        nc.vector.match_replace(eo_sb[:], vals[:, 0:8], eo_sb[:], -1e30)
        nc.vector.max(vals[:, 8:16], eo_sb[:])
        # softmax exp for first 16 values issues while round 3 runs
        nc.scalar.activation(expv[:, 0:16], vals[:, 0:16], AF.Exp, bias=negv0[:], scale=0.25)
        den1 = pool.tile([128, 1], f32, tag="den1")
        nc.vector.tensor_reduce(den1[:], expv[:, 0:16], AX.X, AL.add)
        nc.vector.match_replace(eo_sb[:], vals[:, 8:16], eo_sb[:], -1e30)
        nc.vector.max(vals[:, 16:24], eo_sb[:])
        nc.scalar.activation(expv[:, 16:24], vals[:, 16:24], AF.Exp, bias=negv0[:], scale=0.25)
        # preload Sin activation table behind a dummy op (after last Exp)
        sin_dummy = pool.tile([1, 1], f32, tag="sdum")
        nc.scalar.activation(sin_dummy[:], one_sb[:], AF.Sin, scale=0.1)

        # delay via index search (A/B: v2-style max_index + arithmetic remap)
        idx8 = pool.tile([128, 8], u32, tag="idx8")
        nc.vector.max_index(idx8[:], vals[:, 16:24], eo_sb[:])
        c_i = pool.tile([128, 1], i32, tag="ci")
        nc.vector.tensor_copy(c_i[:], idx8[:, 5:6].bitcast(i32))
        c_neg = pool.tile([128, 1], i32, tag="cneg")
        nc.vector.tensor_scalar(c_neg[:], c_i[:], -1, 3072, AL.mult, AL.add)
        mask = pool.tile([128, 1], i32, tag="mask")
        nc.vector.tensor_scalar(mask[:], c_i[:], 1024, None, AL.is_gt)
        d_sel = pool.tile([128, 1], i32, tag="dsel")
        nc.vector.select(d_sel[:], mask[:], c_neg[:], c_i[:])
        d_f = pool.tile([128, 1], f32, tag="df")
        nc.vector.tensor_copy(d_f[:], d_sel[:])

        # ---------------- phases (sw = phs[0], cw = phs[1]; wgt folded into wo_a) ----------------
        # sw-arg = (j*d) mod 2048 ; cw-arg = (sw-arg + 512) mod 2048 (int chain;
        # arith ops cast on write, bitVec must keep dtype)
        mi0 = pool2.tile([128, NF], i32, tag="pmi")
        mi1 = pool2.tile([128, NF], i32, tag="pmi")
        mf0 = pool2.tile([128, NF], f16, tag="pmf")   # args <= 2047: exact in fp16
        mf1 = pool2.tile([128, NF], f16, tag="pmf")
        # j*d (+off) on the scalar engine (exact ints < 2^21), mod+downcast on vector
        nc.scalar.activation(mi0[:], io_f[:], AF.Identity, scale=d_f[:])
        nc.scalar.activation(mi1[:], io_f[:], AF.Identity, scale=d_f[:], bias=b512[:])
        nc.vector.tensor_scalar(mi0[:], mi0[:], 2047, None, AL.bitwise_and)
        nc.vector.tensor_copy(mf0[:], mi0[:])
        nc.vector.tensor_scalar(mi1[:], mi1[:], 2047, None, AL.bitwise_and)
        nc.vector.tensor_copy(mf1[:], mi1[:])
        nc.scalar.activation(mf0[:], mf0[:], AF.Sin,
                             scale=float(np.pi / 1024.0), bias=negpi[:])
        nc.scalar.activation(mf1[:], mf1[:], AF.Sin,
                             scale=float(np.pi / 1024.0), bias=negpi[:])
        phs = [mf0[:], mf1[:]]
        # softmax weight; fold (wgt * -2/T) into wo -> wo_a / won_a
        den = pool.tile([128, 1], f32, tag="den")
        nc.vector.tensor_reduce(den[:], expv[:, 16:22], AX.X, AL.add)
        nc.vector.tensor_add(den[:], den[:], den1[:])
        rden = pool.tile([128, 1], f32, tag="rden")
        nc.vector.reciprocal(rden[:], den[:])
        wgt = pool.tile([128, 1], f32, tag="wgt")
        nc.vector.tensor_mul(wgt[:], expv[:, 21:22], rden[:])
        alpha = pool.tile([128, 1], f32, tag="alpha")
        nc.vector.tensor_scalar_mul(alpha[:], wgt[:], -2.0 / T * 64.0)
        wo_a = pool.tile([128, H * E], f16, tag="woa")
        won_a = pool.tile([128, H * E], f16, tag="wona")
        nc.vector.tensor_scalar(wo_a[:], wo_sb[:], alpha[:], None, AL.mult)
        nc.vector.tensor_scalar(won_a[:], wo_a[:], -1.0, None, AL.mult)
        sw, cw = phs[0], phs[1]
        # nyquist scale: (1-2*(d&1)) * wgt / T
        d_i = pool.tile([128, 1], i32, tag="di")
        nc.vector.tensor_copy(d_i[:], d_f[:])
        par_i = pool.tile([128, 1], i32, tag="par")
        nc.vector.tensor_scalar(par_i[:], d_i[:], 1, None, AL.bitwise_and)
        parf = pool.tile([128, 1], f32, tag="parf")
        nc.vector.tensor_copy(parf[:], par_i[:])
        nc.vector.tensor_scalar(parf[:], parf[:], -2.0, 1.0, AL.mult, AL.add)
        nys = pool.tile([128, 1], f32, tag="nys")
        nc.vector.tensor_scalar(nys[:], parf[:], wgt[:], 1.0 / T, AL.mult, AL.mult)

        # ---------------- phase multiply + output projection (wo_a stationary) ----------------
        gn_cols = pool.tile([128, H], f16, tag="gncols")
        og_re = pqk.tile([128, 1024], f32, tag="qk1024")
        og_im = pqk.tile([128, 1024], f32, tag="qk1024")
        for h in range(H):
            vre, vim = vre_t[h], vim_t[h]
            nc.vector.tensor_scalar(gn_cols[:, h:h + 1], vn_cols[:, h:h + 1], nys[:], None, AL.mult)
            m1 = pool2.tile([128, NF], f16, tag="m1")
            m2 = pool2.tile([128, NF], f16, tag="m2")
            m3 = pool2.tile([128, NF], f16, tag="m3")
            m4 = pool2.tile([128, NF], f16, tag="m4")
            nc.vector.tensor_tensor(m1[:], vre[:], cw, AL.mult)
            nc.vector.tensor_tensor(m2[:], vim[:], sw, AL.mult)
            nc.vector.tensor_tensor(m3[:], vre[:], sw, AL.mult)
            nc.vector.tensor_tensor(m4[:], vim[:], cw, AL.mult)
            for s in range(2):
                sl = slice(s * 512, (s + 1) * 512)
                nc.tensor.matmul(og_re[:, sl], wo_a[:, h * E:(h + 1) * E], m1[:, sl],
                                 start=(h == 0), stop=False)
                nc.tensor.matmul(og_re[:, sl], won_a[:, h * E:(h + 1) * E], m2[:, sl],
                                 start=False, stop=(h == H - 1))
                nc.tensor.matmul(og_im[:, sl], wo_a[:, h * E:(h + 1) * E], m3[:, sl],
                                 start=(h == 0), stop=False)
                nc.tensor.matmul(og_im[:, sl], wo_a[:, h * E:(h + 1) * E], m4[:, sl],
                                 start=False, stop=(h == H - 1))
        ofn_ps = prow.tile([1, E], f32, tag="row")
        for h in range(H):
            nc.tensor.matmul(ofn_ps[:], gn_cols[:, h:h + 1], wo_sb[:, h * E:(h + 1) * E],
                             start=(h == 0), stop=(h == H - 1))
        ofn_row = pool.tile([1, E], f16, tag="ofnrow")
        nc.vector.tensor_copy(ofn_row[:], ofn_ps[:])

        # transpose og [e'', f] -> of [f, e''] via PE (f32r); og->g2 on vector (idle)
        g2_re = pool.tile([128, 1024], f32r, tag="g2re")
        g2_im = pool.tile([128, 1024], f32r, tag="g2im")
        nc.vector.tensor_copy(g2_re[:], og_re[:])
        nc.vector.tensor_copy(g2_im[:], og_im[:])
        of_re = pool.tile([128, NCH * E], f16, tag="ofre")
        of_im = pool.tile([128, NCH * E], f16, tag="ofim")
        id_r = id_sb[:]
        for half in range(4):
            tp = pb.tile([128, 512], f32, tag="b512")
            src = g2_re if half < 2 else g2_im
            dst = of_re if half < 2 else of_im
            base = (half % 2) * 512
            for q in range(4):
                j = (half % 2) * 4 + q
                nc.tensor.transpose(tp[:, q * 128:(q + 1) * 128].bitcast(f32r),
                                    src[:, j * 128:(j + 1) * 128], id_r)
            nc.scalar.copy(dst[:, base:base + 512], tp[:])

        # t = 1024 output row first so its DMA never gates the teardown
        o1_ps = prow.tile([128, 1], f32, tag="row")
        for j in range(NCH):
            nc.tensor.matmul(o1_ps[:], of_re[:, j * E:(j + 1) * E], altp_sb[:],
                             start=(j == 0), stop=False)
        nc.tensor.matmul(o1_ps[:], ofn_row[:], one_sb[:], start=False, stop=False)
        nc.tensor.matmul(o1_ps[:], of_re[0:1, 0:E], mhalf_sb[:], start=False, stop=True)
        o1_sb = pool.tile([128, 1], f32, tag="o1sb")
        nc.vector.tensor_scalar(o1_sb[:], o1_ps[:], bo_sb[:], None, AL.add)
        nc.sync.dma_start(o1024_d[:], o1_sb[:])

        # final inverse per s-half: op first (feeds scalar), then ep; combine via
        # scalar (op+bo / -op+bo straight from PSUM) + vector add of ep
        for s in range(2):
            sl = slice(s * 512, (s + 1) * 512)
            op_ps = pb.tile([128, 512], f32, tag="b512")
            ep_ps = pb.tile([128, 512], f32, tag="b512")
            for j in range(NCH):
                nc.tensor.matmul(op_ps[:], of_im[:, j * E:(j + 1) * E],
                                 sn_sb[:, j * NF + s * 512: j * NF + (s + 1) * 512],
                                 start=(j == 0), stop=(j == NCH - 1))
            for j in range(NCH):
                nc.tensor.matmul(ep_ps[:], of_re[:, j * E:(j + 1) * E],
                                 cs_sb[:, j * NF + s * 512: j * NF + (s + 1) * 512],
                                 start=(j == 0), stop=False)
            nc.tensor.matmul(ep_ps[:], ofn_row[:], altf_sb[:, sl], start=False, stop=False)
            nc.tensor.matmul(ep_ps[:], of_re[0:1, 0:E], mhrow_sb[:, sl], start=False, stop=True)
            tpos = pool2.tile([128, 512], f32, tag="tpos", bufs=1)
            tneg = pool2.tile([128, 512], f32, tag="tneg", bufs=1)
            nc.scalar.activation(tpos[:], op_ps[:], AF.Identity, bias=bo_sb[:], scale=1.0)
            nc.scalar.activation(tneg[:], op_ps[:], AF.Identity, bias=bo_sb[:], scale=-1.0)
            out_lo = pool2.tile([128, 512], f32, tag="outlo")
            out_hi = pool2.tile([128, 512], f32, tag="outhi")
            nc.vector.tensor_add(out_lo[:], ep_ps[:], tneg[:])
            nc.vector.tensor_add(out_hi[:], ep_ps[:], tpos[:])
            nc.sync.dma_start(lo_d[:, sl], out_lo[:])
            nc.sync.dma_start(hi_d[:, sl], out_hi[:])

    nc.compile()
    return nc


def _get_nc():
    if "nc" not in _CACHE:
        _wire_ntff_hook()
        _CACHE["nc"] = _build()
    return _CACHE["nc"]


def kernel(hidden_states, wq, bq, wk, bk, wv, bv, wo, bo):
    global LAST_EXEC_NS
    nc = _get_nc()
    consts = _CACHE.setdefault("consts", _host_consts())

    def chunked(a):
        # [1024, W] -> [128, 8*W] with chunk-major columns (device layout)
        W = a.shape[1]
        return np.ascontiguousarray(
            a.reshape(NCH, 128, W).transpose(1, 0, 2).reshape(128, NCH * W))

    hs = np.ascontiguousarray(hidden_states, dtype=np.float32)
    wqk = np.ascontiguousarray(
        (np.concatenate([wq.transpose(2, 0, 1), wk.transpose(2, 0, 1)], axis=2)
         * (1.0 / 32.0)).transpose(1, 0, 2).reshape(128, H * 256)).astype(np.float16)
    wv_h = np.ascontiguousarray(
        wv.transpose(2, 0, 1).transpose(1, 0, 2).reshape(128, H * E)).astype(np.float16)
    wo_h = np.ascontiguousarray(
        wo.transpose(1, 0, 2).transpose(1, 0, 2).reshape(128, H * E)).astype(np.float16)
    # interleaved per head: [q_h (128) | k_h (128)] * 4, scaled (T/32)
    bqk = np.ascontiguousarray(
        np.stack([(T / 32.0) * bq.T, (T / 32.0) * bk.T], axis=1).reshape(-1)[None, :]
    ).astype(np.float32)                                                     # [1, 2*H*E]
    bv_s = np.ascontiguousarray(T * bv, dtype=np.float32)                     # [E, H]
    bo_c = np.ascontiguousarray(bo, dtype=np.float32)[:, None]                # [E, 1]

    in_maps = []
    for b in range(B):
        x = hs[b]
        xr = np.concatenate([x[0:1], x[:0:-1]])[:NF]
        xc = (x[:NF] + xr)
        xc[0] *= 0.5
        xs = (xr - x[:NF])
        in_maps.append({
            "xc": chunked(xc).astype(np.float16), "xs": chunked(xs).astype(np.float16),
            "xnyq": x[NF:NF + 1].astype(np.float16),
            "cs": consts["cs"], "sn": consts["sn"], "altf": consts["altf"],
            "altp": consts["altp"], "one": consts["one"], "mhalf": consts["mhalf"],
            "mhrow": consts["mhrow"], "ident": consts["ident"],
            "wqk": wqk, "wv": wv_h, "wo": wo_h, "bqk": bqk, "bv": bv_s, "bo": bo_c,
        })

    trace = bool(int(os.environ.get("BASS_KERNEL_TRACE", "0")))
    res = run_bass_kernel_spmd(nc, in_maps, core_ids=list(range(B)), trace=trace)
    LAST_EXEC_NS = res.exec_time_ns
    _CACHE["last_res"] = res

    out = np.empty((B, T, E), dtype=np.float32)
    for b in range(B):
        r = res.results[b]
        out[b, 0:NF] = r["out_lo"].T
        out[b, NF] = r["out_1024"][:, 0]
        out[b, NF + 1:] = r["out_hi"][:, 1:NF][:, ::-1].T
    return out
